# revision 22
# baseline (speedup 1.0000x reference)
"""Trainium2 Bass kernel for nn_DecoderAttentionRNN (Show-Attend-Tell decoder).

Data-parallel over batch M=128 -> 16 rows per core on 8 NeuronCores.
Per core: phase 0 (enc_a projection, embedding projection), phase 1
(39 sequential attention+LSTM steps), phase 2 (batched vocab projection
over all 39 stored hidden states).
"""
import sys, types, os

sys.path.insert(0, '/opt/trn_rl_repo')


def _install_ntff_hook():
    try:
        import antenv
    except Exception:
        return
    if 'antenv.axon_hooks' in sys.modules:
        return
    mod = types.ModuleType('antenv.axon_hooks')
    _state = {'hook': None}

    def set_axon_ntff_profile_hook(h):
        _state['hook'] = h

    def get_axon_ntff_profile_hook():
        if _state['hook'] is None:
            try:
                from trn_agent_boot.trn_boot import _ntff_profile_via_ctypes
                _state['hook'] = _ntff_profile_via_ctypes('/opt/axon/libaxon_pjrt.so')
            except Exception:
                _state['hook'] = None
        return _state['hook']

    mod.set_axon_ntff_profile_hook = set_axon_ntff_profile_hook
    mod.get_axon_ntff_profile_hook = get_axon_ntff_profile_hook
    sys.modules['antenv.axon_hooks'] = mod
    antenv.axon_hooks = mod


_install_ntff_hook()

import numpy as np
import ml_dtypes

import concourse.bass as bass
import concourse.bacc as bacc
import concourse.tile as tile
import concourse.mybir as mybir
from concourse import bass_utils

BF16 = mybir.dt.bfloat16
F32 = mybir.dt.float32
NP_BF16 = ml_dtypes.bfloat16

N_CORES = 8
M, MC = 128, 16          # batch, batch per core
L, D = 196, 1280         # attention positions (14*14), encoder dim
K, H, E, G = 512, 512, 512, 2048  # attn dim, hidden, embed, 4H
V, S, T = 10000, 40, 39  # vocab, seq len, decode steps
LLO, LHI = 128, 68       # l-chunks 0:128, 128:196
NS_EA = 448              # enc_a free-dim split (3136 = 7*448)
ML = MC * L              # 3136

AluOp = mybir.AluOpType
ActFn = mybir.ActivationFunctionType


def _vsplits(total, step):
    out = []
    o = 0
    while o < total:
        out.append((o, min(step, total - o)))
        o += step
    return out


def build_program(trace_label=None):
    nc = bacc.Bacc("TRN2", target_bir_lowering=False, debug=False,
                   num_devices=N_CORES)
    dt = nc.dram_tensor

    # ---- per-core external inputs ----
    enc_lo_d = dt("enc_lo", (LLO, MC * D), BF16, kind="ExternalInput").ap()
    enc_hi_d = dt("enc_hi", (LHI, MC * D), BF16, kind="ExternalInput").ap()
    encT_d = dt("encT", (D, ML), BF16, kind="ExternalInput").ap()
    h0T_d = dt("h0T", (128, 4 * MC), BF16, kind="ExternalInput").ap()
    c0_d = dt("c0", (MC, H), F32, kind="ExternalInput").ap()
    embsT_d = dt("embsT", (E + 1, T * MC), BF16, kind="ExternalInput").ap()
    W_iheT_d = dt("W_iheT", (E + 1, G), BF16, kind="ExternalInput").ap()
    W_decT_d = dt("W_decT", (128, 16 * 128), BF16, kind="ExternalInput").ap()
    W_attT_d = dt("W_attT", (128, 4), BF16, kind="ExternalInput").ap()
    W_betaT_d = dt("W_betaT", (128, 4), BF16, kind="ExternalInput").ap()
    W_encT_d = dt("W_encT", (128, 40 * 128), BF16, kind="ExternalInput").ap()
    b_ed_d = dt("b_ed", (128, 4), F32, kind="ExternalInput").ap()
    W_xhT_d = dt("W_xhT", (D + H, G), BF16, kind="ExternalInput").ap()
    W_fcT_d = dt("W_fcT", (H, V), BF16, kind="ExternalInput").ap()
    mask39_d = dt("mask39", (MC, T), F32, kind="ExternalInput").ap()
    mask624_d = dt("mask624", (128, 5), F32, kind="ExternalInput").ap()
    Wattm_d = dt("Wattm", (128, 4 * MC * MC), BF16, kind="ExternalInput").ap()
    id_bf_d = dt("id_bf", (128, 128), BF16, kind="ExternalInput").ap()
    id_f32_d = dt("id_f32", (128, 128), F32, kind="ExternalInput").ap()
    ones_d = dt("ones_bf", (1, 128), BF16, kind="ExternalInput").ap()
    b_beta_d = dt("b_beta", (MC, 1), F32, kind="ExternalInput").ap()

    # ---- per-core external outputs ----
    scores_d = dt("scores", (T * MC, V), F32, kind="ExternalOutput").ap()
    alphas_d = dt("alphas", (T, MC, L), F32, kind="ExternalOutput").ap()

    with tile.TileContext(nc) as tc:
        # outer pool: survives all phases
        with tc.tile_pool(name="persist", bufs=1) as pp:
            ENC_lo = pp.tile([128, MC * D], BF16)
            ENC_hi = pp.tile([128, MC * D], BF16)
            enc_aT = pp.tile([128, 4 * ML], BF16)     # kc-blocks of [128, 3136]
            EMBd = pp.tile([T * MC, G], BF16, space="DRAM")  # emb-proj scratch
            hT_all = pp.tile([128, 4 * T * MC], BF16)  # kc-blocks of [128, 624]
            h0T = pp.tile([128, 4 * MC], BF16)
            W_dec = pp.tile([128, 16 * 128], BF16)
            W_att = pp.tile([128, 4], BF16)
            Wattm = pp.tile([128, 4 * MC * MC], BF16)
            W_beta = pp.tile([128, 4], BF16)
            b_ed = pp.tile([128, 4], F32)
            id_bf = pp.tile([128, 128], BF16)
            id_f32 = pp.tile([128, 128], F32)
            ones_bf = pp.tile([1, 128], BF16)
            mask39 = pp.tile([MC, T], F32)
            mask624 = pp.tile([128, 5], F32)
            b_beta_t = pp.tile([MC, 1], F32)

            nc.sync.dma_start(ENC_lo[:], enc_lo_d[:, :])
            nc.sync.dma_start(ENC_hi[0:LHI, :], enc_hi_d[:, :])
            nc.sync.dma_start(h0T[:], h0T_d[:, :])
            nc.sync.dma_start(W_dec[:], W_decT_d[:, :])
            nc.sync.dma_start(W_att[:], W_attT_d[:, :])
            nc.sync.dma_start(Wattm[:], Wattm_d[:, :])
            nc.sync.dma_start(W_beta[:], W_betaT_d[:, :])
            nc.sync.dma_start(b_ed[:], b_ed_d[:, :])
            nc.sync.dma_start(mask39[:], mask39_d[:, :])
            nc.sync.dma_start(mask624[:], mask624_d[:, :])
            nc.sync.dma_start(id_bf[:], id_bf_d[:, :])
            nc.sync.dma_start(id_f32[:], id_f32_d[:, :])
            nc.sync.dma_start(ones_bf[:], ones_d[:, :])
            nc.sync.dma_start(b_beta_t[:], b_beta_d[:, :])

            # ============ PHASE 0a: enc_aT = W_enc @ encT (+bias) ============
            with tc.tile_pool(name="p0a_sb", bufs=2) as sp0, \
                 tc.tile_pool(name="p0a_w", bufs=1) as wp0, \
                 tc.tile_pool(name="p0a_ps", bufs=8, space="PSUM") as pp0:
                W_enc_s = wp0.tile([128, 40 * 128], BF16)
                nc.sync.dma_start(W_enc_s[:], W_encT_d[:, :])
                ns_all = _vsplits(ML, NS_EA)  # 7 x 448
                for kcp in (0, 1):            # kc pairs (0,1), (2,3)
                    for nsh in (0, 1):        # ns halves 0:4, 4:7
                        nss = ns_all[:4] if nsh == 0 else ns_all[4:]
                        c0_, c1_ = nss[0][0], nss[-1][0] + nss[-1][1]
                        ncols = c1_ - c0_
                        psl = [[pp0.tile([128, NS_EA], F32, tag="ea_ps", name="ea_ps") for _ in nss]
                               for _ in range(2)]
                        for dc in range(10):
                            et = sp0.tile([128, 1792], BF16, tag="encT")
                            nc.sync.dma_start(
                                et[:, 0:ncols],
                                encT_d[dc * 128:(dc + 1) * 128, c0_:c1_])
                            for ki in range(2):
                                kc = kcp * 2 + ki
                                for si, (so, sn) in enumerate(nss):
                                    nc.tensor.matmul(
                                        psl[ki][si][:],
                                        W_enc_s[:, (dc * 4 + kc) * 128:(dc * 4 + kc + 1) * 128],
                                        et[:, so - c0_:so - c0_ + sn],
                                        start=(dc == 0), stop=(dc == 9))
                        for ki in range(2):
                            kc = kcp * 2 + ki
                            for si, (so, sn) in enumerate(nss):
                                nc.vector.tensor_scalar(
                                    enc_aT[:, kc * ML + so:kc * ML + so + sn],
                                    psl[ki][si][:], b_ed[:, kc:kc + 1], None,
                                    op0=AluOp.add)

            # ============ PHASE 0b: EMBp = embsT.T @ W_iheT (+bias row) ======
            with tc.tile_pool(name="p0b_sb", bufs=2) as sp1, \
                 tc.tile_pool(name="p0b_l", bufs=1) as lp1, \
                 tc.tile_pool(name="p0b_ps", bufs=4, space="PSUM") as pp1:
                embL = lp1.tile([128, 4 * T * MC], BF16)
                embO = lp1.tile([1, T * MC], BF16)
                nc.sync.dma_start(
                    embL[:].rearrange("p (c m) -> p c m", c=4),
                    embsT_d[0:512, :].rearrange("(c p) m -> p c m", p=128))
                nc.sync.dma_start(embO[:], embsT_d[512:513, :])
                wt = [lp1.tile([128, G], BF16, tag=f"wihe{i}", name=f"wihe{i}") for i in range(4)]
                for kc in range(4):
                    nc.sync.dma_start(wt[kc][:],
                                      W_iheT_d[kc * 128:(kc + 1) * 128, :])
                wO = lp1.tile([1, G], BF16)
                nc.sync.dma_start(wO[:], W_iheT_d[512:513, :])
                for mc in range(5):
                    r0, rn = mc * 128, min(128, T * MC - mc * 128)
                    psg = [pp1.tile([128, 512], F32, tag="p0b_ps", name="p0b_ps") for _ in range(4)]
                    for kc in range(4):
                        for n in range(4):
                            nc.tensor.matmul(
                                psg[n][0:rn, :],
                                embL[:, kc * T * MC + r0:kc * T * MC + r0 + rn],
                                wt[kc][:, n * 512:(n + 1) * 512],
                                start=(kc == 0), stop=False)
                    for n in range(4):
                        nc.tensor.matmul(
                            psg[n][0:rn, :], embO[:, r0:r0 + rn],
                            wO[:, n * 512:(n + 1) * 512],
                            start=False, stop=True)
                    embstg = sp1.tile([128, G], BF16, tag="embstg")
                    for n in range(4):
                        nc.vector.tensor_copy(
                            embstg[0:rn, n * 512:(n + 1) * 512],
                            psg[n][0:rn, :])
                    nc.sync.dma_start(EMBd[r0:r0 + rn, :], embstg[0:rn, :])

            # ============ PHASE 1: recurrence ============
            with tc.tile_pool(name="c_pool", bufs=2) as cpool, \
                 tc.tile_pool(name="step", bufs=2) as stp, \
                 tc.tile_pool(name="step1", bufs=1) as stp1, \
                 tc.tile_pool(name="x_pool", bufs=1) as xp, \
                 tc.tile_pool(name="wihd", bufs=4) as wdp, \
                 tc.tile_pool(name="ps_small", bufs=2, space="PSUM") as pss, \
                 tc.tile_pool(name="ps_awe", bufs=1, space="PSUM") as psa, \
                 tc.tile_pool(name="ps_gA", bufs=1, space="PSUM") as psg_pool:

                Xt = xp.tile([128, 4 * ML], BF16)  # relu buffer, kc-blocks
                c_prev = cpool.tile([MC, H], F32)
                nc.sync.dma_start(c_prev[:], c0_d[:, :])

                for t in range(T):
                    if t == 0:
                        hT_prev = [h0T[:, kc * MC:(kc + 1) * MC]
                                   for kc in range(4)]
                    else:
                        hT_prev = [hT_all[:, kc * T * MC + (t - 1) * MC:
                                          kc * T * MC + t * MC]
                                   for kc in range(4)]

                    # --- dec_aT [128(k), 4kc x 16m] ---
                    ps_dec = pss.tile([128, 4 * MC], F32, tag="sm")
                    for kc in range(4):
                        for hh in range(4):
                            nc.tensor.matmul(
                                ps_dec[:, kc * MC:(kc + 1) * MC],
                                W_dec[:, (hh * 4 + kc) * 128:(hh * 4 + kc + 1) * 128],
                                hT_prev[hh],
                                start=(hh == 0), stop=(hh == 3))
                    dec_sb = stp1.tile([128, 4 * MC], F32, tag="dec")
                    nc.vector.tensor_copy(dec_sb[:], ps_dec[:])

                    # --- X = relu(enc_aT + dec_a) ---
                    for kc in range(4):
                        for m in range(MC):
                            src = enc_aT[:, kc * ML + m * L:kc * ML + (m + 1) * L]
                            dst = Xt[:, kc * ML + m * L:kc * ML + (m + 1) * L]
                            bias = dec_sb[:, kc * MC + m:kc * MC + m + 1]
                            if m < 8:
                                nc.vector.tensor_scalar(
                                    dst, src, bias, 0.0,
                                    op0=AluOp.add, op1=AluOp.max)
                            else:
                                nc.scalar.activation(dst, src, ActFn.Relu,
                                                     bias=bias)

                    # --- e[m, l]: masked-column lhsT, all m accumulate ---
                    ps_e = pss.tile([MC, L], F32, tag="sm")
                    for kc in range(4):
                        for m in range(MC):
                            blk = kc * MC + m
                            nc.tensor.matmul(
                                ps_e[:],
                                Wattm[:, blk * MC:(blk + 1) * MC],
                                Xt[:, kc * ML + m * L:kc * ML + (m + 1) * L],
                                start=(kc == 0 and m == 0),
                                stop=(kc == 3 and m == MC - 1),
                                skip_group_check=True)

                    # --- softmax ---
                    nmx = stp1.tile([MC, 1], F32, tag="nmx")
                    nc.vector.tensor_reduce(nmx[:], ps_e[:], mybir.AxisListType.X,
                                            AluOp.max, negate=True)
                    exp_sb = stp1.tile([MC, L], F32, tag="exp")
                    ssum = stp1.tile([MC, 1], F32, tag="ssum")
                    nc.scalar.activation(exp_sb[:], ps_e[:], ActFn.Exp,
                                         bias=nmx[:], accum_out=ssum[:])
                    rs = stp1.tile([MC, 1], F32, tag="rs")
                    nc.vector.reciprocal(rs[:], ssum[:])

                    # --- beta ---
                    ps_b = pss.tile([MC, 1], F32, tag="sm")
                    for hh in range(4):
                        nc.tensor.matmul(ps_b[:], hT_prev[hh],
                                         W_beta[:, hh:hh + 1],
                                         start=(hh == 0), stop=(hh == 3))
                    beta_sb = stp1.tile([MC, 1], F32, tag="beta")
                    nc.scalar.activation(beta_sb[:], ps_b[:], ActFn.Sigmoid,
                                         bias=b_beta_t[:])

                    # --- alpha (awe path, x beta) and alphas output (x mask) ---
                    al_awe = stp.tile([MC, L], BF16, tag="al_awe")
                    nc.vector.tensor_scalar(al_awe[:], exp_sb[:], rs[:],
                                            beta_sb[:], op0=AluOp.mult,
                                            op1=AluOp.mult)
                    al_out = stp1.tile([MC, L], F32, tag="al_out")
                    nc.vector.tensor_scalar(al_out[:], exp_sb[:], rs[:],
                                            mask39[:, t:t + 1], op0=AluOp.mult,
                                            op1=AluOp.mult)
                    nc.sync.dma_start(alphas_d[t], al_out[:])

                    # --- alphaT (masked columns) via PE transpose + col copies ---
                    alT_msk = stp1.tile([128, 2 * MC * MC], BF16, tag="alT")
                    nc.vector.memset(alT_msk[:], 0.0)
                    ps_t1 = pss.tile([128, MC], BF16, tag="sm")
                    nc.tensor.transpose(ps_t1[:], al_awe[:, 0:128],
                                        id_bf[0:MC, 0:MC])
                    for m in range(MC):
                        nc.vector.tensor_copy(
                            alT_msk[:, m * MC + m:m * MC + m + 1],
                            ps_t1[:, m:m + 1])
                    ps_t2 = pss.tile([LHI, MC], BF16, tag="sm")
                    nc.tensor.transpose(ps_t2[:], al_awe[:, 128:196],
                                        id_bf[0:MC, 0:MC])
                    for m in range(MC):
                        blk = MC + m
                        nc.vector.tensor_copy(
                            alT_msk[0:LHI, blk * MC + m:blk * MC + m + 1],
                            ps_t2[:, m:m + 1])

                    # --- awe[m, d] = sum_l alpha[m,l] enc[m,l,d]  (x beta) ---
                    ps_awe = psa.tile([MC, D], F32)
                    for (so, sn) in _vsplits(D, 512):
                        for m in range(MC):
                            nc.tensor.matmul(
                                ps_awe[:, so:so + sn],
                                alT_msk[:, m * MC:(m + 1) * MC],
                                ENC_lo[:, m * D + so:m * D + so + sn],
                                start=(m == 0), stop=False,
                                skip_group_check=True)
                            nc.tensor.matmul(
                                ps_awe[:, so:so + sn],
                                alT_msk[0:LHI, (MC + m) * MC:(MC + m + 1) * MC],
                                ENC_hi[0:LHI, m * D + so:m * D + so + sn],
                                start=False, stop=(m == MC - 1),
                                skip_group_check=True)
                    awe_sb = stp1.tile([MC, D], BF16, tag="awe")
                    nc.scalar.copy(awe_sb[:], ps_awe[:])

                    # --- x_inDT via PE transposes ---
                    xdT = stp1.tile([128, 10 * MC], BF16, tag="xdT")
                    for dc in range(10):
                        ps_tx = pss.tile([128, MC], BF16, tag="sm")
                        nc.tensor.transpose(ps_tx[:],
                                            awe_sb[:, dc * 128:(dc + 1) * 128],
                                            id_bf[0:MC, 0:MC])
                        nc.vector.tensor_copy(xdT[:, dc * MC:(dc + 1) * MC],
                                              ps_tx[:])

                    # --- gates ---
                    emb_t = stp1.tile([MC, G], BF16, tag="emb_t")
                    nc.sync.dma_start(emb_t[:], EMBd[t * MC:(t + 1) * MC, :])
                    emb_rhs = emb_t
                    ps_gA = psg_pool.tile([MC, 3 * 512], F32)
                    ps_gB = pss.tile([MC, 512], F32, tag="sm")
                    for n in range(3):
                        nc.tensor.matmul(ps_gA[:, n * 512:(n + 1) * 512],
                                         id_bf[0:MC, 0:MC],
                                         emb_rhs[:, n * 512:(n + 1) * 512],
                                         start=True, stop=False,
                                         skip_group_check=True)
                    nc.tensor.matmul(ps_gB[:], id_bf[0:MC, 0:MC],
                                     emb_rhs[:, 3 * 512:4 * 512],
                                     start=True, stop=False,
                                     skip_group_check=True)
                    for dc in range(14):
                        wd = wdp.tile([128, G], BF16, tag="wihd")
                        nc.sync.dma_start(wd[:],
                                          W_xhT_d[dc * 128:(dc + 1) * 128, :])
                        lhs = (xdT[:, dc * MC:(dc + 1) * MC] if dc < 10
                               else hT_prev[dc - 10])
                        last = (dc == 13)
                        for n in range(3):
                            nc.tensor.matmul(ps_gA[:, n * 512:(n + 1) * 512],
                                             lhs, wd[:, n * 512:(n + 1) * 512],
                                             start=False, stop=last,
                                             skip_group_check=True)
                        nc.tensor.matmul(ps_gB[:], lhs,
                                         wd[:, 3 * 512:4 * 512],
                                         start=False, stop=last,
                                         skip_group_check=True)

                    # --- LSTM pointwise ---
                    sig_if = stp1.tile([MC, 1024], F32, tag="sig_if")
                    nc.scalar.activation(sig_if[:], ps_gA[:, 0:1024],
                                         ActFn.Sigmoid)
                    tg = stp1.tile([MC, 512], F32, tag="tg")
                    nc.scalar.activation(tg[:], ps_gA[:, 1024:1536], ActFn.Tanh)
                    so_ = stp1.tile([MC, 512], F32, tag="so")
                    nc.scalar.activation(so_[:], ps_gB[:], ActFn.Sigmoid)
                    t1 = stp1.tile([MC, 512], F32, tag="t1")
                    nc.vector.tensor_mul(t1[:], sig_if[:, 512:1024], c_prev[:])
                    t2 = stp1.tile([MC, 512], F32, tag="t2")
                    nc.vector.tensor_mul(t2[:], sig_if[:, 0:512], tg[:])
                    c_new = cpool.tile([MC, H], F32, tag="c_prev")
                    nc.vector.tensor_add(c_new[:], t1[:], t2[:])
                    tc_ = stp1.tile([MC, 512], F32, tag="tc")
                    nc.scalar.activation(tc_[:], c_new[:], ActFn.Tanh)
                    h_new = stp1.tile([MC, H], F32, tag="h_new")
                    nc.vector.tensor_mul(h_new[:], so_[:], tc_[:])

                    # --- hT via PE transpose (fp32 -> bf16) ---
                    for kc in range(4):
                        ps_th = pss.tile([128, MC], F32, tag="sm")
                        nc.tensor.transpose(ps_th[:],
                                            h_new[:, kc * 128:(kc + 1) * 128],
                                            id_f32[0:MC, 0:MC])
                        nc.vector.tensor_copy(
                            hT_all[:, kc * T * MC + t * MC:
                                   kc * T * MC + (t + 1) * MC],
                            ps_th[:])
                    c_prev = c_new

            # ============ PHASE 2: scores = H @ W_fc.T (b_fc added on host) ==
            VH = V // 2
            with tc.tile_pool(name="p2_w", bufs=1) as wp2, \
                 tc.tile_pool(name="p2_st", bufs=4) as sp2, \
                 tc.tile_pool(name="p2_ps", bufs=8, space="PSUM") as pp2:
                for vh in range(2):
                    W_fch = wp2.tile([128, 4 * VH], BF16, tag="wfch")
                    nc.sync.dma_start(
                        W_fch[:].rearrange("p (c v) -> p c v", c=4),
                        W_fcT_d[:, vh * VH:(vh + 1) * VH]
                        .rearrange("(c p) v -> p c v", p=128))
                    nv_all = _vsplits(VH, 512)  # 9x512 + 392
                    for mc in range(5):
                        r0, rn = mc * 128, min(128, T * MC - mc * 128)
                        for g0 in range(0, len(nv_all), 5):
                            grp = nv_all[g0:g0 + 5]
                            psl = [pp2.tile([128, 512], F32, tag="p2ps", name="p2ps") for _ in grp]
                            for kc in range(4):
                                lhs = hT_all[:, kc * T * MC + r0:
                                             kc * T * MC + r0 + rn]
                                for gi, (vo, vn) in enumerate(grp):
                                    nc.tensor.matmul(
                                        psl[gi][0:rn, 0:vn], lhs,
                                        W_fch[:, kc * VH + vo:kc * VH + vo + vn],
                                        start=(kc == 0), stop=(kc == 3))
                            for gi, (vo, vn) in enumerate(grp):
                                st = sp2.tile([128, 512], F32, tag="out")
                                nc.vector.tensor_scalar(
                                    st[0:rn, 0:vn], psl[gi][0:rn, 0:vn],
                                    mask624[0:rn, mc:mc + 1], None,
                                    op0=AluOp.mult)
                                nc.sync.dma_start(
                                    scores_d[r0:r0 + rn,
                                             vh * VH + vo:vh * VH + vo + vn],
                                    st[0:rn, 0:vn])

    nc.compile()
    return nc


_PROGRAM_CACHE = {}


def _get_program():
    if 'nc' not in _PROGRAM_CACHE:
        _PROGRAM_CACHE['nc'] = build_program()
    return _PROGRAM_CACHE['nc']


def _prep_inputs(inputs):
    """Host-side prep: sorting, gathers, init states, per-core shards."""
    enc_full = np.asarray(inputs['encoder_out'], np.float32).reshape(M, L, D)
    caps_in = np.asarray(inputs['encoded_captions'])
    lens = np.asarray(inputs['caption_lengths']).reshape(M).astype(np.int64)
    W_enc = np.asarray(inputs['W_enc'], np.float32)
    b_enc = np.asarray(inputs['b_enc'], np.float32)
    W_dec = np.asarray(inputs['W_dec'], np.float32)
    b_dec = np.asarray(inputs['b_dec'], np.float32)
    W_att = np.asarray(inputs['W_att'], np.float32)
    emb = np.asarray(inputs['emb'], np.float32)
    W_ih = np.asarray(inputs['W_ih'], np.float32)
    b_ih = np.asarray(inputs['b_ih'], np.float32)
    W_hh = np.asarray(inputs['W_hh'], np.float32)
    b_hh = np.asarray(inputs['b_hh'], np.float32)
    W_h0 = np.asarray(inputs['W_h0'], np.float32)
    b_h0 = np.asarray(inputs['b_h0'], np.float32)
    W_c0 = np.asarray(inputs['W_c0'], np.float32)
    b_c0 = np.asarray(inputs['b_c0'], np.float32)
    W_beta = np.asarray(inputs['W_beta'], np.float32)
    b_beta = np.asarray(inputs['b_beta'], np.float32)
    W_fc = np.asarray(inputs['W_fc'], np.float32)
    b_fc = np.asarray(inputs['b_fc'], np.float32)

    sorted_idx = np.argsort(-lens, kind='stable').astype(np.int32)
    decode_lengths = (lens[sorted_idx] - 1).astype(np.int32)
    caps = caps_in[sorted_idx].astype(np.int32)
    enc = enc_full[sorted_idx]
    embs = emb[caps[:, :T]]                       # [M, T, E]
    mean_enc = enc.mean(axis=1)
    h0 = mean_enc @ W_h0.T + b_h0
    c0 = mean_enc @ W_c0.T + b_c0
    active = (np.arange(T)[None, :] < decode_lengths[:, None]).astype(np.float32)

    # shared (replicated) weight blobs
    W_decT = W_dec.T.copy()                       # [H, K]
    wdec_blk = np.zeros((128, 16 * 128), np.float32)
    for hh in range(4):
        for kc in range(4):
            b = hh * 4 + kc
            wdec_blk[:, b * 128:(b + 1) * 128] = \
                W_decT[hh * 128:(hh + 1) * 128, kc * 128:(kc + 1) * 128]
    W_attT = W_att[0].reshape(4, 128).T.copy()    # [128, 4]
    Wattm = np.zeros((128, 4 * MC * MC), np.float32)
    for kc in range(4):
        for m in range(MC):
            blk = kc * MC + m
            Wattm[:, blk * MC + m] = W_attT[:, kc]
    W_betaT = W_beta[0].reshape(4, 128).T.copy()
    W_encT_blk = np.zeros((128, 40 * 128), np.float32)
    W_encT_full = W_enc.T.copy()                  # [D, K]
    for dc in range(10):
        for kc in range(4):
            b = dc * 4 + kc
            W_encT_blk[:, b * 128:(b + 1) * 128] = \
                W_encT_full[dc * 128:(dc + 1) * 128, kc * 128:(kc + 1) * 128]
    b_ed = (b_enc + b_dec).reshape(4, 128).T.copy()  # [128, 4]
    W_iheT = np.concatenate([W_ih[:, :E].T, (b_ih + b_hh)[None, :]], axis=0)
    W_xhT = np.concatenate([W_ih[:, E:].T, W_hh.T], axis=0)  # [D+H, G]
    W_fcT = W_fc.T.copy()                         # [H, V]

    eye = np.eye(128, dtype=np.float32)
    shared = {
        'id_bf': eye.astype(NP_BF16),
        'id_f32': eye,
        'ones_bf': np.ones((1, 128), NP_BF16),
        'b_beta': np.full((MC, 1), float(b_beta.reshape(-1)[0]), np.float32),
        'W_decT': wdec_blk.astype(NP_BF16),
        'W_attT': W_attT.astype(NP_BF16),
        'Wattm': Wattm.astype(NP_BF16),
        'W_betaT': W_betaT.astype(NP_BF16),
        'W_encT': W_encT_blk.astype(NP_BF16),
        'b_ed': b_ed.astype(np.float32),
        'W_iheT': W_iheT.astype(NP_BF16),
        'W_xhT': W_xhT.astype(NP_BF16),
        'W_fcT': W_fcT.astype(NP_BF16),
    }

    in_maps = []
    for c in range(N_CORES):
        ms = slice(c * MC, (c + 1) * MC)
        enc_c = enc[ms]                            # [16, 196, 1280]
        enc_lo = enc_c[:, :LLO].transpose(1, 0, 2).reshape(LLO, MC * D)
        enc_hi = enc_c[:, LLO:].transpose(1, 0, 2).reshape(LHI, MC * D)
        encT = enc_c.transpose(2, 0, 1).reshape(D, ML)
        h0T_c = h0[ms].T.reshape(4, 128, MC).transpose(1, 0, 2).reshape(128, 4 * MC)
        embsT = embs[ms].transpose(2, 1, 0).reshape(E, T * MC)
        embsT_aug = np.concatenate([embsT, np.ones((1, T * MC), np.float32)], 0)
        act_c = active[ms]                         # [16, 39]
        m624 = np.zeros((128, 5), np.float32)
        r = np.arange(T * MC)
        flat = act_c[r % MC, r // MC]              # row r = t*16+m
        for mc in range(5):
            rn = min(128, T * MC - mc * 128)
            m624[:rn, mc] = flat[mc * 128:mc * 128 + rn]
        im = {
            'enc_lo': enc_lo.astype(NP_BF16),
            'enc_hi': enc_hi.astype(NP_BF16),
            'encT': encT.astype(NP_BF16),
            'h0T': h0T_c.astype(NP_BF16),
            'c0': c0[ms].astype(np.float32),
            'embsT': embsT_aug.astype(NP_BF16),
            'mask39': act_c.astype(np.float32),
            'mask624': m624,
        }
        im.update(shared)
        in_maps.append(im)

    return in_maps, sorted_idx, decode_lengths, caps, active


def kernel(_trace=False, **inputs):
    in_maps, sorted_idx, decode_lengths, caps, active = _prep_inputs(inputs)
    b_fc_host = np.asarray(inputs['b_fc'], np.float32)
    nc = _get_program()
    res = bass_utils.run_bass_kernel_spmd(
        nc, in_maps, core_ids=list(range(N_CORES)), trace=_trace)

    pred_scores = np.empty((M, T, V), np.float32)
    alphas = np.empty((M, T, L), np.float32)
    for c in range(N_CORES):
        ms = slice(c * MC, (c + 1) * MC)
        sc = res.results[c]['scores'].reshape(T, MC, V)
        pred_scores[ms] = sc.transpose(1, 0, 2)
        al = res.results[c]['alphas'].reshape(T, MC, L)
        alphas[ms] = al.transpose(1, 0, 2)
    pred_scores += (active[:, :, None] * b_fc_host[None, None, :])

    if _trace:
        kernel._last_results = res
    return pred_scores, caps, decode_lengths, alphas, sorted_idx


# revision 26
# speedup vs baseline: 1.0225x; 1.0225x over previous
"""Trainium2 Bass kernel for nn_DecoderAttentionRNN (Show-Attend-Tell decoder).

Data-parallel over batch M=128 -> 16 rows per core on 8 NeuronCores.
Per core: phase 0 (enc_a projection, embedding projection), phase 1
(39 sequential attention+LSTM steps), phase 2 (batched vocab projection
over all 39 stored hidden states).
"""
import sys, types, os

sys.path.insert(0, '/opt/trn_rl_repo')


def _install_ntff_hook():
    try:
        import antenv
    except Exception:
        return
    if 'antenv.axon_hooks' in sys.modules:
        return
    mod = types.ModuleType('antenv.axon_hooks')
    _state = {'hook': None}

    def set_axon_ntff_profile_hook(h):
        _state['hook'] = h

    def get_axon_ntff_profile_hook():
        if _state['hook'] is None:
            try:
                from trn_agent_boot.trn_boot import _ntff_profile_via_ctypes
                _state['hook'] = _ntff_profile_via_ctypes('/opt/axon/libaxon_pjrt.so')
            except Exception:
                _state['hook'] = None
        return _state['hook']

    mod.set_axon_ntff_profile_hook = set_axon_ntff_profile_hook
    mod.get_axon_ntff_profile_hook = get_axon_ntff_profile_hook
    sys.modules['antenv.axon_hooks'] = mod
    antenv.axon_hooks = mod


_install_ntff_hook()

import numpy as np
import ml_dtypes

import concourse.bass as bass
import concourse.bacc as bacc
import concourse.tile as tile
import concourse.mybir as mybir
from concourse import bass_utils

# Let walrus optimize LDWEIGHTS scheduling/elision (default-off in this repo).
_orig_run_command = bass_utils.run_command


def _patched_run_command(argv, **kw):
    argv = [a
            for a in argv]
    return _orig_run_command(argv, **kw)


bass_utils.run_command = _patched_run_command

BF16 = mybir.dt.bfloat16
F32 = mybir.dt.float32
NP_BF16 = ml_dtypes.bfloat16

N_CORES = 8
M, MC = 128, 16          # batch, batch per core
L, D = 196, 1280         # attention positions (14*14), encoder dim
K, H, E, G = 512, 512, 512, 2048  # attn dim, hidden, embed, 4H
V, S, T = 10000, 40, 39  # vocab, seq len, decode steps
LLO, LHI = 128, 68       # l-chunks 0:128, 128:196
NS_EA = 448              # enc_a free-dim split (3136 = 7*448)
ML = MC * L              # 3136

AluOp = mybir.AluOpType
ActFn = mybir.ActivationFunctionType


def _vsplits(total, step):
    out = []
    o = 0
    while o < total:
        out.append((o, min(step, total - o)))
        o += step
    return out


def build_program(trace_label=None):
    nc = bacc.Bacc("TRN2", target_bir_lowering=False, debug=False,
                   num_devices=N_CORES)
    dt = nc.dram_tensor

    # ---- per-core external inputs ----
    enc_lo_d = dt("enc_lo", (LLO, MC * D), BF16, kind="ExternalInput").ap()
    enc_hi_d = dt("enc_hi", (LHI, MC * D), BF16, kind="ExternalInput").ap()
    encT_d = dt("encT", (D, ML), BF16, kind="ExternalInput").ap()
    h0T_d = dt("h0T", (128, 4 * MC), BF16, kind="ExternalInput").ap()
    c0_d = dt("c0", (MC, H), F32, kind="ExternalInput").ap()
    embsT_d = dt("embsT", (E + 1, T * MC), BF16, kind="ExternalInput").ap()
    W_iheT_d = dt("W_iheT", (E + 1, G), BF16, kind="ExternalInput").ap()
    W_decT_d = dt("W_decT", (128, 16 * 128), BF16, kind="ExternalInput").ap()
    W_attT_d = dt("W_attT", (128, 4), BF16, kind="ExternalInput").ap()
    W_betaT_d = dt("W_betaT", (128, 4), BF16, kind="ExternalInput").ap()
    W_encT_d = dt("W_encT", (128, 40 * 128), BF16, kind="ExternalInput").ap()
    b_ed_d = dt("b_ed", (128, 4), F32, kind="ExternalInput").ap()
    W_xhT_d = dt("W_xhT", (D + H, G), BF16, kind="ExternalInput").ap()
    W_fcT_d = dt("W_fcT", (H, V), BF16, kind="ExternalInput").ap()
    mask39_d = dt("mask39", (MC, T), F32, kind="ExternalInput").ap()
    mask624_d = dt("mask624", (128, 5), F32, kind="ExternalInput").ap()
    Wattm_d = dt("Wattm", (128, 4 * MC * MC), BF16, kind="ExternalInput").ap()
    id_bf_d = dt("id_bf", (128, 128), BF16, kind="ExternalInput").ap()
    id_f32_d = dt("id_f32", (128, 128), F32, kind="ExternalInput").ap()
    ones_d = dt("ones_bf", (1, 128), BF16, kind="ExternalInput").ap()
    b_beta_d = dt("b_beta", (MC, 1), F32, kind="ExternalInput").ap()

    # ---- per-core external outputs ----
    scores_d = dt("scores", (T * MC, V), F32, kind="ExternalOutput").ap()
    alphas_d = dt("alphas", (T, MC, L), F32, kind="ExternalOutput").ap()

    with tile.TileContext(nc) as tc:
        # outer pool: survives all phases
        with tc.tile_pool(name="persist", bufs=1) as pp:
            ENC_lo = pp.tile([128, MC * D], BF16)
            ENC_hi = pp.tile([128, MC * D], BF16)
            enc_aT = pp.tile([128, 4 * ML], BF16)     # kc-blocks of [128, 3136]
            EMBd = pp.tile([T * MC, G], BF16, space="DRAM")  # emb-proj scratch
            hT_all = pp.tile([128, 4 * T * MC], BF16)  # kc-blocks of [128, 624]
            h0T = pp.tile([128, 4 * MC], BF16)
            W_dec = pp.tile([128, 16 * 128], BF16)
            W_att = pp.tile([128, 4], BF16)
            Wattm = pp.tile([128, 4 * MC * MC], BF16)
            W_beta = pp.tile([128, 4], BF16)
            b_ed = pp.tile([128, 4], F32)
            id_bf = pp.tile([128, 128], BF16)
            id_f32 = pp.tile([128, 128], F32)
            ones_bf = pp.tile([1, 128], BF16)
            mask39 = pp.tile([MC, T], F32)
            mask624 = pp.tile([128, 5], F32)
            b_beta_t = pp.tile([MC, 1], F32)

            nc.sync.dma_start(ENC_lo[:], enc_lo_d[:, :])
            nc.sync.dma_start(ENC_hi[0:LHI, :], enc_hi_d[:, :])
            nc.sync.dma_start(h0T[:], h0T_d[:, :])
            nc.sync.dma_start(W_dec[:], W_decT_d[:, :])
            nc.sync.dma_start(W_att[:], W_attT_d[:, :])
            nc.sync.dma_start(Wattm[:], Wattm_d[:, :])
            nc.sync.dma_start(W_beta[:], W_betaT_d[:, :])
            nc.sync.dma_start(b_ed[:], b_ed_d[:, :])
            nc.sync.dma_start(mask39[:], mask39_d[:, :])
            nc.sync.dma_start(mask624[:], mask624_d[:, :])
            nc.sync.dma_start(id_bf[:], id_bf_d[:, :])
            nc.sync.dma_start(id_f32[:], id_f32_d[:, :])
            nc.sync.dma_start(ones_bf[:], ones_d[:, :])
            nc.sync.dma_start(b_beta_t[:], b_beta_d[:, :])

            # ============ PHASE 0a: enc_aT = W_enc @ encT (+bias) ============
            with tc.tile_pool(name="p0a_sb", bufs=2) as sp0, \
                 tc.tile_pool(name="p0a_w", bufs=1) as wp0, \
                 tc.tile_pool(name="p0a_ps", bufs=8, space="PSUM") as pp0:
                W_enc_s = wp0.tile([128, 40 * 128], BF16)
                nc.sync.dma_start(W_enc_s[:], W_encT_d[:, :])
                ns_all = _vsplits(ML, NS_EA)  # 7 x 448
                for kcp in (0, 1):            # kc pairs (0,1), (2,3)
                    for nsh in (0, 1):        # ns halves 0:4, 4:7
                        nss = ns_all[:4] if nsh == 0 else ns_all[4:]
                        c0_, c1_ = nss[0][0], nss[-1][0] + nss[-1][1]
                        ncols = c1_ - c0_
                        psl = [[pp0.tile([128, NS_EA], F32, tag="ea_ps", name="ea_ps") for _ in nss]
                               for _ in range(2)]
                        for dc in range(10):
                            et = sp0.tile([128, 1792], BF16, tag="encT")
                            nc.sync.dma_start(
                                et[:, 0:ncols],
                                encT_d[dc * 128:(dc + 1) * 128, c0_:c1_])
                            for ki in range(2):
                                kc = kcp * 2 + ki
                                for si, (so, sn) in enumerate(nss):
                                    nc.tensor.matmul(
                                        psl[ki][si][:],
                                        W_enc_s[:, (dc * 4 + kc) * 128:(dc * 4 + kc + 1) * 128],
                                        et[:, so - c0_:so - c0_ + sn],
                                        start=(dc == 0), stop=(dc == 9))
                        for ki in range(2):
                            kc = kcp * 2 + ki
                            for si, (so, sn) in enumerate(nss):
                                nc.vector.tensor_scalar(
                                    enc_aT[:, kc * ML + so:kc * ML + so + sn],
                                    psl[ki][si][:], b_ed[:, kc:kc + 1], None,
                                    op0=AluOp.add)

            # ============ PHASE 0b: EMBp = embsT.T @ W_iheT (+bias row) ======
            with tc.tile_pool(name="p0b_sb", bufs=2) as sp1, \
                 tc.tile_pool(name="p0b_l", bufs=1) as lp1, \
                 tc.tile_pool(name="p0b_ps", bufs=4, space="PSUM") as pp1:
                embL = lp1.tile([128, 4 * T * MC], BF16)
                embO = lp1.tile([1, T * MC], BF16)
                nc.sync.dma_start(
                    embL[:].rearrange("p (c m) -> p c m", c=4),
                    embsT_d[0:512, :].rearrange("(c p) m -> p c m", p=128))
                nc.sync.dma_start(embO[:], embsT_d[512:513, :])
                wt = [lp1.tile([128, G], BF16, tag=f"wihe{i}", name=f"wihe{i}") for i in range(4)]
                for kc in range(4):
                    nc.sync.dma_start(wt[kc][:],
                                      W_iheT_d[kc * 128:(kc + 1) * 128, :])
                wO = lp1.tile([1, G], BF16)
                nc.sync.dma_start(wO[:], W_iheT_d[512:513, :])
                for mc in range(5):
                    r0, rn = mc * 128, min(128, T * MC - mc * 128)
                    psg = [pp1.tile([128, 512], F32, tag="p0b_ps", name="p0b_ps") for _ in range(4)]
                    for kc in range(4):
                        for n in range(4):
                            nc.tensor.matmul(
                                psg[n][0:rn, :],
                                embL[:, kc * T * MC + r0:kc * T * MC + r0 + rn],
                                wt[kc][:, n * 512:(n + 1) * 512],
                                start=(kc == 0), stop=False)
                    for n in range(4):
                        nc.tensor.matmul(
                            psg[n][0:rn, :], embO[:, r0:r0 + rn],
                            wO[:, n * 512:(n + 1) * 512],
                            start=False, stop=True)
                    embstg = sp1.tile([128, G], BF16, tag="embstg")
                    for n in range(4):
                        nc.vector.tensor_copy(
                            embstg[0:rn, n * 512:(n + 1) * 512],
                            psg[n][0:rn, :])
                    nc.sync.dma_start(EMBd[r0:r0 + rn, :], embstg[0:rn, :])

            # ============ PHASE 1: recurrence ============
            with tc.tile_pool(name="c_pool", bufs=2) as cpool, \
                 tc.tile_pool(name="step", bufs=2) as stp, \
                 tc.tile_pool(name="step1", bufs=1) as stp1, \
                 tc.tile_pool(name="x_pool", bufs=1) as xp, \
                 tc.tile_pool(name="wihd", bufs=3) as wdp, \
                 tc.tile_pool(name="ps_small", bufs=2, space="PSUM") as pss, \
                 tc.tile_pool(name="ps_awe", bufs=1, space="PSUM") as psa, \
                 tc.tile_pool(name="ps_gA", bufs=1, space="PSUM") as psg_pool:

                Xt = xp.tile([128, 4 * ML], BF16)  # relu buffer, kc-blocks
                c_prev = cpool.tile([MC, H], F32)
                nc.sync.dma_start(c_prev[:], c0_d[:, :])

                for t in range(T):
                    if t == 0:
                        hT_prev = [h0T[:, kc * MC:(kc + 1) * MC]
                                   for kc in range(4)]
                    else:
                        hT_prev = [hT_all[:, kc * T * MC + (t - 1) * MC:
                                          kc * T * MC + t * MC]
                                   for kc in range(4)]

                    # --- dec_aT [128(k), 4kc x 16m] ---
                    ps_dec = pss.tile([128, 4 * MC], F32, tag="sm")
                    for kc in range(4):
                        for hh in range(4):
                            nc.tensor.matmul(
                                ps_dec[:, kc * MC:(kc + 1) * MC],
                                W_dec[:, (hh * 4 + kc) * 128:(hh * 4 + kc + 1) * 128],
                                hT_prev[hh],
                                start=(hh == 0), stop=(hh == 3))
                    dec_sb = stp1.tile([128, 4 * MC], F32, tag="dec")
                    nc.vector.tensor_copy(dec_sb[:], ps_dec[:])

                    # --- gates psum: eye(EMBproj) + W_hh part early ---
                    emb_t = stp1.tile([MC, G], BF16, tag="emb_t")
                    nc.sync.dma_start(emb_t[:], EMBd[t * MC:(t + 1) * MC, :])
                    ps_gA = psg_pool.tile([MC, 3 * 512], F32)
                    ps_gB = pss.tile([MC, 512], F32, tag="sm")
                    for n in range(3):
                        nc.tensor.matmul(ps_gA[:, n * 512:(n + 1) * 512],
                                         id_bf[0:MC, 0:MC],
                                         emb_t[:, n * 512:(n + 1) * 512],
                                         start=True, stop=False,
                                         skip_group_check=True)
                    nc.tensor.matmul(ps_gB[:], id_bf[0:MC, 0:MC],
                                     emb_t[:, 3 * 512:4 * 512],
                                     start=True, stop=False,
                                     skip_group_check=True)
                    for dc in range(10, 14):
                        wd = wdp.tile([128, G], BF16, tag="wihd")
                        nc.sync.dma_start(wd[:],
                                          W_xhT_d[dc * 128:(dc + 1) * 128, :])
                        lhs = hT_prev[dc - 10]
                        for n in range(3):
                            nc.tensor.matmul(ps_gA[:, n * 512:(n + 1) * 512],
                                             lhs, wd[:, n * 512:(n + 1) * 512],
                                             start=False, stop=False,
                                             skip_group_check=True)
                        nc.tensor.matmul(ps_gB[:], lhs,
                                         wd[:, 3 * 512:4 * 512],
                                         start=False, stop=False,
                                         skip_group_check=True)

                    # --- X = relu(enc_aT + dec_a) ---
                    for kc in range(4):
                        for m in range(MC):
                            src = enc_aT[:, kc * ML + m * L:kc * ML + (m + 1) * L]
                            dst = Xt[:, kc * ML + m * L:kc * ML + (m + 1) * L]
                            bias = dec_sb[:, kc * MC + m:kc * MC + m + 1]
                            if m < 10:
                                nc.vector.tensor_scalar(
                                    dst, src, bias, 0.0,
                                    op0=AluOp.add, op1=AluOp.max)
                            else:
                                nc.scalar.activation(dst, src, ActFn.Relu,
                                                     bias=bias)

                    # --- e[m, l]: masked-column lhsT, all m accumulate ---
                    ps_e = pss.tile([MC, L], F32, tag="sm")
                    for kc in range(4):
                        for m in range(MC):
                            blk = kc * MC + m
                            nc.tensor.matmul(
                                ps_e[:],
                                Wattm[:, blk * MC:(blk + 1) * MC],
                                Xt[:, kc * ML + m * L:kc * ML + (m + 1) * L],
                                start=(kc == 0 and m == 0),
                                stop=(kc == 3 and m == MC - 1),
                                skip_group_check=True)

                    # --- softmax (no max-subtraction: |e| is small) ---
                    exp_sb = stp1.tile([MC, L], F32, tag="exp")
                    ssum = stp1.tile([MC, 1], F32, tag="ssum")
                    nc.scalar.activation(exp_sb[:], ps_e[:], ActFn.Exp,
                                         accum_out=ssum[:])
                    rs = stp1.tile([MC, 1], F32, tag="rs")
                    nc.vector.reciprocal(rs[:], ssum[:])

                    # --- beta ---
                    ps_b = pss.tile([MC, 1], F32, tag="sm")
                    for hh in range(4):
                        nc.tensor.matmul(ps_b[:], hT_prev[hh],
                                         W_beta[:, hh:hh + 1],
                                         start=(hh == 0), stop=(hh == 3))
                    beta_sb = stp1.tile([MC, 1], F32, tag="beta")
                    nc.scalar.activation(beta_sb[:], ps_b[:], ActFn.Sigmoid,
                                         bias=b_beta_t[:])

                    # --- alpha (awe path, x beta) and alphas output (x mask) ---
                    al_awe = stp.tile([MC, L], BF16, tag="al_awe")
                    nc.vector.tensor_scalar(al_awe[:], exp_sb[:], rs[:],
                                            beta_sb[:], op0=AluOp.mult,
                                            op1=AluOp.mult)
                    al_out = stp1.tile([MC, L], F32, tag="al_out")
                    nc.vector.tensor_scalar(al_out[:], exp_sb[:], rs[:],
                                            mask39[:, t:t + 1], op0=AluOp.mult,
                                            op1=AluOp.mult)
                    nc.sync.dma_start(alphas_d[t], al_out[:])

                    # --- alphaT (masked columns) via PE transpose + col copies ---
                    alT_msk = stp.tile([128, 2 * MC * MC], BF16, tag="alT")
                    nc.vector.memset(alT_msk[:], 0.0)
                    ps_t1 = pss.tile([128, MC], BF16, tag="sm")
                    nc.tensor.transpose(ps_t1[:], al_awe[:, 0:128],
                                        id_bf[0:MC, 0:MC])
                    for m in range(MC):
                        nc.vector.tensor_copy(
                            alT_msk[:, m * MC + m:m * MC + m + 1],
                            ps_t1[:, m:m + 1])
                    ps_t2 = pss.tile([LHI, MC], BF16, tag="sm")
                    nc.tensor.transpose(ps_t2[:], al_awe[:, 128:196],
                                        id_bf[0:MC, 0:MC])
                    for m in range(MC):
                        blk = MC + m
                        nc.vector.tensor_copy(
                            alT_msk[0:LHI, blk * MC + m:blk * MC + m + 1],
                            ps_t2[:, m:m + 1])

                    # --- awe[m, d] = sum_l alpha[m,l] enc[m,l,d]  (x beta) ---
                    ps_awe = psa.tile([MC, D], F32)
                    for m in range(MC):
                        for (so, sn) in _vsplits(D, 512):
                            nc.tensor.matmul(
                                ps_awe[:, so:so + sn],
                                alT_msk[:, m * MC:(m + 1) * MC],
                                ENC_lo[:, m * D + so:m * D + so + sn],
                                start=(m == 0), stop=False,
                                skip_group_check=True)
                        for (so, sn) in _vsplits(D, 512):
                            nc.tensor.matmul(
                                ps_awe[:, so:so + sn],
                                alT_msk[0:LHI, (MC + m) * MC:(MC + m + 1) * MC],
                                ENC_hi[0:LHI, m * D + so:m * D + so + sn],
                                start=False, stop=(m == MC - 1),
                                skip_group_check=True)
                    awe_sb = stp1.tile([MC, D], BF16, tag="awe")
                    nc.vector.tensor_copy(awe_sb[:], ps_awe[:])

                    # --- x_inDT via PE transposes ---
                    xdT = stp1.tile([128, 10 * MC], BF16, tag="xdT")
                    for dc in range(10):
                        ps_tx = pss.tile([128, MC], BF16, tag="sm")
                        nc.tensor.transpose(ps_tx[:],
                                            awe_sb[:, dc * 128:(dc + 1) * 128],
                                            id_bf[0:MC, 0:MC])
                        nc.vector.tensor_copy(xdT[:, dc * MC:(dc + 1) * MC],
                                              ps_tx[:])

                    # --- gates: x_in part ---
                    for dc in range(10):
                        wd = wdp.tile([128, G], BF16, tag="wihd")
                        nc.sync.dma_start(wd[:],
                                          W_xhT_d[dc * 128:(dc + 1) * 128, :])
                        lhs = xdT[:, dc * MC:(dc + 1) * MC]
                        last = (dc == 9)
                        for n in range(3):
                            nc.tensor.matmul(ps_gA[:, n * 512:(n + 1) * 512],
                                             lhs, wd[:, n * 512:(n + 1) * 512],
                                             start=False, stop=last,
                                             skip_group_check=True)
                        nc.tensor.matmul(ps_gB[:], lhs,
                                         wd[:, 3 * 512:4 * 512],
                                         start=False, stop=last,
                                         skip_group_check=True)

                    # --- LSTM pointwise ---
                    sig_if = stp1.tile([MC, 1024], F32, tag="sig_if")
                    nc.scalar.activation(sig_if[:], ps_gA[:, 0:1024],
                                         ActFn.Sigmoid)
                    tg = stp1.tile([MC, 512], F32, tag="tg")
                    nc.scalar.activation(tg[:], ps_gA[:, 1024:1536], ActFn.Tanh)
                    so_ = stp1.tile([MC, 512], F32, tag="so")
                    nc.scalar.activation(so_[:], ps_gB[:], ActFn.Sigmoid)
                    t1 = stp1.tile([MC, 512], F32, tag="t1")
                    nc.vector.tensor_mul(t1[:], sig_if[:, 512:1024], c_prev[:])
                    t2 = stp1.tile([MC, 512], F32, tag="t2")
                    nc.vector.tensor_mul(t2[:], sig_if[:, 0:512], tg[:])
                    c_new = cpool.tile([MC, H], F32, tag="c_prev")
                    nc.vector.tensor_add(c_new[:], t1[:], t2[:])
                    tc_ = stp1.tile([MC, 512], F32, tag="tc")
                    nc.scalar.activation(tc_[:], c_new[:], ActFn.Tanh)
                    h_new = stp1.tile([MC, H], F32, tag="h_new")
                    nc.vector.tensor_mul(h_new[:], so_[:], tc_[:])

                    # --- hT via PE transpose (fp32 -> bf16) ---
                    for kc in range(4):
                        ps_th = pss.tile([128, MC], F32, tag="sm")
                        nc.tensor.transpose(ps_th[:],
                                            h_new[:, kc * 128:(kc + 1) * 128],
                                            id_f32[0:MC, 0:MC])
                        nc.vector.tensor_copy(
                            hT_all[:, kc * T * MC + t * MC:
                                   kc * T * MC + (t + 1) * MC],
                            ps_th[:])
                    c_prev = c_new

            # ============ PHASE 2: scores = H @ W_fc.T (b_fc added on host) ==
            VH = V // 2
            with tc.tile_pool(name="p2_w", bufs=1) as wp2, \
                 tc.tile_pool(name="p2_st", bufs=4) as sp2, \
                 tc.tile_pool(name="p2_ps", bufs=8, space="PSUM") as pp2:
                for vh in range(2):
                    W_fch = wp2.tile([128, 4 * VH], BF16, tag="wfch")
                    nc.sync.dma_start(
                        W_fch[:].rearrange("p (c v) -> p c v", c=4),
                        W_fcT_d[:, vh * VH:(vh + 1) * VH]
                        .rearrange("(c p) v -> p c v", p=128))
                    nv_all = _vsplits(VH, 512)  # 9x512 + 392
                    for mc in range(5):
                        r0, rn = mc * 128, min(128, T * MC - mc * 128)
                        for g0 in range(0, len(nv_all), 5):
                            grp = nv_all[g0:g0 + 5]
                            psl = [pp2.tile([128, 512], F32, tag="p2ps", name="p2ps") for _ in grp]
                            for kc in range(4):
                                lhs = hT_all[:, kc * T * MC + r0:
                                             kc * T * MC + r0 + rn]
                                for gi, (vo, vn) in enumerate(grp):
                                    nc.tensor.matmul(
                                        psl[gi][0:rn, 0:vn], lhs,
                                        W_fch[:, kc * VH + vo:kc * VH + vo + vn],
                                        start=(kc == 0), stop=(kc == 3))
                            for gi, (vo, vn) in enumerate(grp):
                                st = sp2.tile([128, 512], F32, tag="out")
                                nc.vector.tensor_scalar(
                                    st[0:rn, 0:vn], psl[gi][0:rn, 0:vn],
                                    mask624[0:rn, mc:mc + 1], None,
                                    op0=AluOp.mult)
                                nc.sync.dma_start(
                                    scores_d[r0:r0 + rn,
                                             vh * VH + vo:vh * VH + vo + vn],
                                    st[0:rn, 0:vn])

    nc.compile()
    return nc


_PROGRAM_CACHE = {}


def _get_program():
    if 'nc' not in _PROGRAM_CACHE:
        _PROGRAM_CACHE['nc'] = build_program()
    return _PROGRAM_CACHE['nc']


def _prep_inputs(inputs):
    """Host-side prep: sorting, gathers, init states, per-core shards."""
    enc_full = np.asarray(inputs['encoder_out'], np.float32).reshape(M, L, D)
    caps_in = np.asarray(inputs['encoded_captions'])
    lens = np.asarray(inputs['caption_lengths']).reshape(M).astype(np.int64)
    W_enc = np.asarray(inputs['W_enc'], np.float32)
    b_enc = np.asarray(inputs['b_enc'], np.float32)
    W_dec = np.asarray(inputs['W_dec'], np.float32)
    b_dec = np.asarray(inputs['b_dec'], np.float32)
    W_att = np.asarray(inputs['W_att'], np.float32)
    emb = np.asarray(inputs['emb'], np.float32)
    W_ih = np.asarray(inputs['W_ih'], np.float32)
    b_ih = np.asarray(inputs['b_ih'], np.float32)
    W_hh = np.asarray(inputs['W_hh'], np.float32)
    b_hh = np.asarray(inputs['b_hh'], np.float32)
    W_h0 = np.asarray(inputs['W_h0'], np.float32)
    b_h0 = np.asarray(inputs['b_h0'], np.float32)
    W_c0 = np.asarray(inputs['W_c0'], np.float32)
    b_c0 = np.asarray(inputs['b_c0'], np.float32)
    W_beta = np.asarray(inputs['W_beta'], np.float32)
    b_beta = np.asarray(inputs['b_beta'], np.float32)
    W_fc = np.asarray(inputs['W_fc'], np.float32)
    b_fc = np.asarray(inputs['b_fc'], np.float32)

    sorted_idx = np.argsort(-lens, kind='stable').astype(np.int32)
    decode_lengths = (lens[sorted_idx] - 1).astype(np.int32)
    caps = caps_in[sorted_idx].astype(np.int32)
    enc = enc_full[sorted_idx]
    embs = emb[caps[:, :T]]                       # [M, T, E]
    mean_enc = enc.mean(axis=1)
    h0 = mean_enc @ W_h0.T + b_h0
    c0 = mean_enc @ W_c0.T + b_c0
    active = (np.arange(T)[None, :] < decode_lengths[:, None]).astype(np.float32)

    # shared (replicated) weight blobs
    W_decT = W_dec.T.copy()                       # [H, K]
    wdec_blk = np.zeros((128, 16 * 128), np.float32)
    for hh in range(4):
        for kc in range(4):
            b = hh * 4 + kc
            wdec_blk[:, b * 128:(b + 1) * 128] = \
                W_decT[hh * 128:(hh + 1) * 128, kc * 128:(kc + 1) * 128]
    W_attT = W_att[0].reshape(4, 128).T.copy()    # [128, 4]
    Wattm = np.zeros((128, 4 * MC * MC), np.float32)
    for kc in range(4):
        for m in range(MC):
            blk = kc * MC + m
            Wattm[:, blk * MC + m] = W_attT[:, kc]
    W_betaT = W_beta[0].reshape(4, 128).T.copy()
    W_encT_blk = np.zeros((128, 40 * 128), np.float32)
    W_encT_full = W_enc.T.copy()                  # [D, K]
    for dc in range(10):
        for kc in range(4):
            b = dc * 4 + kc
            W_encT_blk[:, b * 128:(b + 1) * 128] = \
                W_encT_full[dc * 128:(dc + 1) * 128, kc * 128:(kc + 1) * 128]
    b_ed = (b_enc + b_dec).reshape(4, 128).T.copy()  # [128, 4]
    W_iheT = np.concatenate([W_ih[:, :E].T, (b_ih + b_hh)[None, :]], axis=0)
    W_xhT = np.concatenate([W_ih[:, E:].T, W_hh.T], axis=0)  # [D+H, G]
    W_fcT = W_fc.T.copy()                         # [H, V]

    eye = np.eye(128, dtype=np.float32)
    shared = {
        'id_bf': eye.astype(NP_BF16),
        'id_f32': eye,
        'ones_bf': np.ones((1, 128), NP_BF16),
        'b_beta': np.full((MC, 1), float(b_beta.reshape(-1)[0]), np.float32),
        'W_decT': wdec_blk.astype(NP_BF16),
        'W_attT': W_attT.astype(NP_BF16),
        'Wattm': Wattm.astype(NP_BF16),
        'W_betaT': W_betaT.astype(NP_BF16),
        'W_encT': W_encT_blk.astype(NP_BF16),
        'b_ed': b_ed.astype(np.float32),
        'W_iheT': W_iheT.astype(NP_BF16),
        'W_xhT': W_xhT.astype(NP_BF16),
        'W_fcT': W_fcT.astype(NP_BF16),
    }

    in_maps = []
    for c in range(N_CORES):
        ms = slice(c * MC, (c + 1) * MC)
        enc_c = enc[ms]                            # [16, 196, 1280]
        enc_lo = enc_c[:, :LLO].transpose(1, 0, 2).reshape(LLO, MC * D)
        enc_hi = enc_c[:, LLO:].transpose(1, 0, 2).reshape(LHI, MC * D)
        encT = enc_c.transpose(2, 0, 1).reshape(D, ML)
        h0T_c = h0[ms].T.reshape(4, 128, MC).transpose(1, 0, 2).reshape(128, 4 * MC)
        embsT = embs[ms].transpose(2, 1, 0).reshape(E, T * MC)
        embsT_aug = np.concatenate([embsT, np.ones((1, T * MC), np.float32)], 0)
        act_c = active[ms]                         # [16, 39]
        m624 = np.zeros((128, 5), np.float32)
        r = np.arange(T * MC)
        flat = act_c[r % MC, r // MC]              # row r = t*16+m
        for mc in range(5):
            rn = min(128, T * MC - mc * 128)
            m624[:rn, mc] = flat[mc * 128:mc * 128 + rn]
        im = {
            'enc_lo': enc_lo.astype(NP_BF16),
            'enc_hi': enc_hi.astype(NP_BF16),
            'encT': encT.astype(NP_BF16),
            'h0T': h0T_c.astype(NP_BF16),
            'c0': c0[ms].astype(np.float32),
            'embsT': embsT_aug.astype(NP_BF16),
            'mask39': act_c.astype(np.float32),
            'mask624': m624,
        }
        im.update(shared)
        in_maps.append(im)

    return in_maps, sorted_idx, decode_lengths, caps, active


def kernel(_trace=False, **inputs):
    in_maps, sorted_idx, decode_lengths, caps, active = _prep_inputs(inputs)
    b_fc_host = np.asarray(inputs['b_fc'], np.float32)
    nc = _get_program()
    res = bass_utils.run_bass_kernel_spmd(
        nc, in_maps, core_ids=list(range(N_CORES)), trace=_trace)

    pred_scores = np.empty((M, T, V), np.float32)
    alphas = np.empty((M, T, L), np.float32)
    for c in range(N_CORES):
        ms = slice(c * MC, (c + 1) * MC)
        sc = res.results[c]['scores'].reshape(T, MC, V)
        pred_scores[ms] = sc.transpose(1, 0, 2)
        al = res.results[c]['alphas'].reshape(T, MC, L)
        alphas[ms] = al.transpose(1, 0, 2)
    pred_scores += (active[:, :, None] * b_fc_host[None, None, :])

    if _trace:
        kernel._last_results = res
    return pred_scores, caps, decode_lengths, alphas, sorted_idx
